# revision 54
# baseline (speedup 1.0000x reference)
"""MLA attention kernel for Trainium2 — 8-core tensor-parallel (self-contained).

Sharding: data-parallel over batch (2) x tensor-parallel over head groups
(4 groups of 4 heads) = 8 cores, SPMD (one NEFF, per-core input shards).
Core ci: batch ci//4, heads [4*(ci%4), 4*(ci%4)+4).

Layout/optimization highlights (~546us baseline -> ~440-460us):
  - every weight is host-pre-reshaped into its SBUF-resident partition-major
    layout so it loads as few large DMAs; first x/wkvd chunks land first so
    kv-down starts ~2us in
  - kv-down accumulates in the pv/aux PSUM banks and q-down in the sc banks,
    so the rmsnorm chain never blocks q-down and vice versa
  - rmsnorm: ACT Square reads the PSUM accumulators directly (parallel with
    the DVE bf16 cast), mean-square via ones-matmul, Sqrt table preloaded at
    kernel start -> the kv AllGather triggers as soon as the barrier clears
  - k-rope is gathered RAW and rotated post-gather, off the collective
    critical path; big wqu/wout weight DMAs are emitted after the collective
    triggers so the triggers' DMA-lane waits don't queue behind them
  - q latent is gathered in ONE fp8e4 AllGather (half the wire bytes);
    A1/A2 are weight-stationary over SBUF-resident all-chunk gathers
  - attention scores: fp8 DoubleRow matmuls over [nope(128) | rope(64+zero)]
    packed k/q operands (kpack/qpack) -> one matmul per 128-key chunk at
    2x rate; rope rows 64-127 of qpack are zeroed so kpack's garbage rows
    never contribute (NaN x 0 = NaN, so BOTH operands' pads are zeroed)
  - causal diagonal band computed at variable width (512/384/256/128) per
    128-key chunk; the leading 128-wide triangles are zeroed on DVE after
    exp (no PE mask matmuls); denominators via all-ones stationary matmul
  - out-projection interleaved per query chunk, output written fp16, final
    head-group reduction done host-side in fp32
  - total error (bf16 + fp8 q/score path) measured 1.26e-2 absmax-rel vs
    the 2e-2 gate; shadow-validated before hardware
"""

import math

import numpy as np
import ml_dtypes

# ---- problem constants (from the reference model) ----
B, S, HID = 2, 2048, 2048
H, D_NOPE, D_ROPE, V_DIM = 16, 128, 64, 128
KV_RANK, Q_RANK = 512, 1536
HEAD_DIM = D_NOPE + D_ROPE
THETA, EPS = 10000.0, 1e-6
NCORES = 8
NH = 4                    # heads per core
T = 512                   # token chunk
NT = S // T
QC = 512                  # attention query chunk
NQC = S // QC
KH = HID // 128           # 16 k-chunks over HID
RQ = Q_RANK // 128        # 12 chunks over q rank
RKV = KV_RANK // 128      # 4 chunks over kv rank
QG = 6                    # q-down rank groups (2 rank-chunks each)
Q_FP8 = True              # gather the q latent in fp8e4
SCALE = 1.0 / math.sqrt(HEAD_DIM)

_CACHE = {}


def build_nc():
    """Build the Bass/Tile program (one NeuronCore, run SPMD on 8)."""
    from contextlib import ExitStack

    import concourse.mybir as mybir
    import concourse.tile as tile
    from concourse import bacc
    from concourse.bass import ds

    dt = mybir.dt
    AF = mybir.ActivationFunctionType
    bf16 = dt.bfloat16
    f32 = dt.float32
    f16 = dt.float16
    f8 = dt.float8e4 if Q_FP8 else dt.bfloat16

    nc = bacc.Bacc(
        "TRN2",
        target_bir_lowering=False,
        debug=False,
        enable_asserts=False,
        num_devices=NCORES,
    )

    TL = S // 4  # local token quarter

    # ---- I/O (all partition-major, host pre-reshaped) ----
    x_ap = nc.dram_tensor("x", [128, KH, TL], bf16, kind="ExternalInput").ap()
    wqd_ap = nc.dram_tensor("wqd", [128, QG, KH, 256], bf16, kind="ExternalInput").ap()
    wqu_ap = nc.dram_tensor("wqu", [128, RQ, NH * HEAD_DIM], f8, kind="ExternalInput").ap()
    wkvd_ap = nc.dram_tensor("wkvd", [128, KH, KV_RANK + D_ROPE], bf16, kind="ExternalInput").ap()
    wkvuk_ap = nc.dram_tensor("wkvuk", [128, RKV, NH * D_NOPE], bf16, kind="ExternalInput").ap()
    wkvuv_ap = nc.dram_tensor("wkvuv", [128, RKV, NH * V_DIM], bf16, kind="ExternalInput").ap()
    wout_ap = nc.dram_tensor("wout", [128, NH, HID], bf16, kind="ExternalInput").ap()
    cos_ap = nc.dram_tensor("cosq", [128, S], bf16, kind="ExternalInput").ap()
    sin_ap = nc.dram_tensor("sinq", [128, S], bf16, kind="ExternalInput").ap()
    tri_ap = nc.dram_tensor("trimask", [128, 128], bf16, kind="ExternalInput").ap()
    ones_ap = nc.dram_tensor("ones128", [128, 128], bf16, kind="ExternalInput").ap()
    out_ap = nc.dram_tensor("out", [S, HID], f16, kind="ExternalOutput").ap()

    with tile.TileContext(nc) as tc, ExitStack() as ctx:
        # ---- PSUM pools: 2x[128,1024] + 2x[128,512] + 2x[128,512] = 8 banks
        sc_ps = ctx.enter_context(tc.tile_pool(name="sc_ps", bufs=2, space="PSUM"))
        pv_ps = ctx.enter_context(tc.tile_pool(name="pv_ps", bufs=2, space="PSUM"))
        aux_ps = ctx.enter_context(tc.tile_pool(name="aux_ps", bufs=2, space="PSUM"))

        const = ctx.enter_context(tc.tile_pool(name="const", bufs=1))
        woutp = ctx.enter_context(tc.tile_pool(name="woutp", bufs=1))
        dram = ctx.enter_context(tc.tile_pool(name="dram", bufs=1, space="DRAM"))

        # ---- up-projection weights (outlive w1 -> allocated below it)
        w2 = tc.alloc_tile_pool(name="w2", bufs=1)
        wkvuk_sb = w2.tile([128, RKV, NH * D_NOPE], bf16, tag="wkvuk")
        wkvuv_sb = w2.tile([128, RKV, NH * V_DIM], bf16, tag="wkvuv")
        wqu_sb = w2.tile([128, RQ, NH * HEAD_DIM], f8, tag="wqu")

        # ---- phase-A0 weights + x (released after A0)
        w1 = tc.alloc_tile_pool(name="w1", bufs=1)
        xt = w1.tile([128, KH, TL], bf16, tag="xt")
        wkvd_sb = w1.tile([128, KH, KV_RANK + D_ROPE], bf16, tag="wkvd")
        wqd_sb = w1.tile([128, QG, KH, 256], bf16, tag="wqd")

        # first chunks land first so kv-down starts immediately
        nc.sync.dma_start(out=xt[:, 0:2, :], in_=x_ap[:, 0:2, :])
        nc.scalar.dma_start(out=wkvd_sb[:, 0:2, :], in_=wkvd_ap[:, 0:2, :])
        nc.sync.dma_start(out=xt[:, 2:4, :], in_=x_ap[:, 2:4, :])
        nc.scalar.dma_start(out=wkvd_sb[:, 2:4, :], in_=wkvd_ap[:, 2:4, :])
        for q4 in range(1, 4):
            hk = ds(q4 * (KH // 4), KH // 4)
            nc.sync.dma_start(out=xt[:, hk, :], in_=x_ap[:, hk, :])
            nc.scalar.dma_start(out=wkvd_sb[:, hk, :], in_=wkvd_ap[:, hk, :])
        for g in range(QG):
            nc.sync.dma_start(out=wqd_sb[:, g, :, :], in_=wqd_ap[:, g, :, :])

        # ---- resident constants (scalar queue, small-first)
        ones_sb = const.tile([128, 128], bf16, name="ones_sb")
        nc.scalar.dma_start(out=ones_sb[:], in_=ones_ap[:])
        nc.scalar.dma_start(out=wkvuk_sb[:], in_=wkvuk_ap[:])
        nc.scalar.dma_start(out=wkvuv_sb[:], in_=wkvuv_ap[:])
        cos_sb = const.tile([128, S], bf16, name="cos_sb")
        nc.scalar.dma_start(out=cos_sb[:], in_=cos_ap[:])
        sin_sb = const.tile([128, S], bf16, name="sin_sb")
        nc.scalar.dma_start(out=sin_sb[:], in_=sin_ap[:])
        tri_sb = const.tile([128, 128], bf16, name="tri_sb")
        nc.scalar.dma_start(out=tri_sb[:], in_=tri_ap[:])
        wout_sb = woutp.tile([128, NH, HID], bf16, tag="wout")

        krope2_sb = const.tile([64, S], bf16, name="krope2_sb")
        at_sb = const.tile([128, NH, S], bf16, name="at_sb")
        eps_sb = const.tile([128, 1], f32, name="eps_sb")
        nc.gpsimd.memset(eps_sb[:], EPS)
        # preload the sqrt ACT table set off the critical path
        warm = const.tile([128, 1], f32, name="warm_sb")
        nc.scalar.activation(warm, eps_sb[:], AF.Sqrt)

        # DRAM bounce buffers for the latent gathers (partition-major; the
        # kv block 4 carries the row-duplicated rotated k-rope)
        gin_kv = dram.tile([128, RKV + 1, TL], bf16, name="gin_kv")
        gout_kv = dram.tile([4, 128, RKV + 1, TL], bf16, name="gout_kv")
        gin_q = dram.tile([128, RQ, TL], f8, name="gin_q")
        gout_q = dram.tile([4, 128, RQ, TL], f8, name="gout_q")
        GROUPS = [[0, 1, 2, 3], [4, 5, 6, 7]]

        # ================= phase A0: local down-projections =================
        # kv-down accumulates in pv/aux banks; krope+mean-square share one
        # sc tile's two banks; q-down rotates through the sc pool.
        wa = tc.alloc_tile_pool(name="wa", bufs=2)
        kv_ps = [
            pv_ps.tile([128, TL], f32, tag="pv", name="kvps0"),
            pv_ps.tile([128, TL], f32, tag="pv", name="kvps1"),
            aux_ps.tile([128, TL], f32, tag="aux", name="kvps2"),
            aux_ps.tile([128, TL], f32, tag="aux", name="kvps3"),
        ]
        a0t = sc_ps.tile([128, 1024], f32, tag="sc", name="a0t")
        krp_ps = a0t[0:64, 0:512]
        ms_ps = a0t[:, 512:1024]
        for k in range(KH):
            for j in range(RKV):
                nc.tensor.matmul(
                    kv_ps[j], wkvd_sb[:, k, ds(j * 128, 128)], xt[:, k, :],
                    start=(k == 0), stop=(k == KH - 1),
                )
            nc.tensor.matmul(
                krp_ps, wkvd_sb[:, k, ds(KV_RANK, D_ROPE)], xt[:, k, :],
                start=(k == 0), stop=(k == KH - 1),
            )
        # rmsnorm: bf16 copy (DVE) + square (ACT, direct from PSUM) -> ones
        # matmul -> Rsqrt (table already resident) -> scale
        kvc_bf = wa.tile([128, RKV, TL], bf16, tag="kvc", bufs=1)
        sq_bf = wa.tile([128, RKV, TL], bf16, tag="sq", bufs=1)
        for j in range(RKV):
            nc.vector.tensor_copy(kvc_bf[:, j, :], kv_ps[j])
            nc.scalar.activation(sq_bf[:, j, :], kv_ps[j], AF.Square)
        for j in range(RKV):
            nc.tensor.matmul(
                ms_ps, ones_sb[:], sq_bf[:, j, :],
                start=(j == 0), stop=(j == RKV - 1),
            )
        srt = wa.tile([128, TL], f32, tag="srt", bufs=1)
        nc.scalar.activation(srt, ms_ps, AF.Sqrt, bias=eps_sb[:], scale=1.0 / KV_RANK)
        rinv = wa.tile([128, TL], f32, tag="rinv", bufs=1)
        nc.vector.reciprocal_approx_fast(out=rinv, in_=srt)
        kvcn = wa.tile([128, RKV, TL], bf16, tag="kvcn", bufs=1)
        for j in range(RKV):
            nc.vector.tensor_mul(kvcn[:, j, :], kvc_bf[:, j, :], rinv)
        # k rope gathered RAW (rows 0-63 only); rotation happens post-gather,
        # off the collective critical path
        krb = wa.tile([64, TL], bf16, tag="krb", bufs=1)
        nc.vector.tensor_copy(krb, krp_ps)
        nc.scalar.dma_start(out=gin_kv[0:64, RKV, :], in_=krb[:])
        nc.scalar.dma_start(out=gin_kv[:, 0:RKV, :], in_=kvcn[:])
        nc.gpsimd.collective_compute(
            "AllGather", mybir.AluOpType.bypass, replica_groups=GROUPS,
            ins=[gin_kv.opt()], outs=[gout_kv.opt()],
        )

        # ---- q down: 6 groups of 2 rank-chunks, each on one sc tile; the
        # fp8 latent is staged per group and gathered in ONE AllGather ----
        qlat8 = wa.tile([128, RQ, TL], f8, tag="qlat8", bufs=1)
        for g in range(QG):
            qt = sc_ps.tile([128, 1024], f32, tag="sc", name="qdt")
            for k in range(KH):
                for m in range(2):
                    nc.tensor.matmul(
                        qt[:, ds(m * 512, 512)],
                        wqd_sb[:, g, k, ds(m * 128, 128)], xt[:, k, :],
                        start=(k == 0), stop=(k == KH - 1),
                    )
            for m in range(2):
                nc.vector.tensor_copy(qlat8[:, 2 * g + m, :], qt[:, ds(m * 512, 512)])
            nc.scalar.dma_start(
                out=gin_q[:, ds(2 * g, 2), :], in_=qlat8[:, ds(2 * g, 2), :]
            )
        nc.gpsimd.collective_compute(
            "AllGather", mybir.AluOpType.bypass, replica_groups=GROUPS,
            ins=[gin_q.opt()], outs=[gout_q.opt()],
        )
        # big late-needed weights AFTER the collective triggers, so the
        # trigger instructions' DMA-lane waits don't queue behind them
        nc.sync.dma_start(out=wqu_sb[:], in_=wqu_ap[:])

        wa.release()
        w1.release()
        # SBUF-resident intermediates (reuse w1's region)
        kvsb = tc.alloc_tile_pool(name="kvsb", bufs=1)
        # fp8 score operands, DoubleRow-packed: [.., kc, {nope,rope}, 128] /
        # [.., {nope,rope}, token]; rope rows 64-127 of qpack are ZERO so the
        # garbage rows of kpack's rope half never contribute
        kpack = kvsb.tile([128, NH, S // 128, 2, 128], f8, tag="kpack")
        qpack = kvsb.tile([128, NH, 2, S], f8, tag="qpack")
        v_sb = kvsb.tile([128, S // 128, NH * V_DIM], bf16, tag="v")
        nc.gpsimd.memset(qpack[64:128, :, 1, :], 0.0)
        nc.gpsimd.memset(kpack[64:128, :, :, 1, :], 0.0)
        gat = tc.alloc_tile_pool(name="gat", bufs=1)
        kvg = gat.tile([128, RKV, NT, T], bf16, tag="kvg")
        qlg = gat.tile([128, RQ, NT, T], f8, tag="qlg")
        wb = tc.alloc_tile_pool(name="wb", bufs=2)

        # ================= phase A1: kv up-projections (weight-stationary) ===
        for c in range(NT):
            nc.scalar.dma_start(out=kvg[:, :, c, :], in_=gout_kv[c, :, 0:RKV, :])
        # k-rope rotation for the full sequence (raw gathered halves):
        # [x1;x2] rows 0-63, shifted copy, rotate, duplicate to rows 64-127
        krraw = wb.tile([64, S], bf16, tag="krraw", bufs=1)
        for c in range(NT):
            nc.scalar.dma_start(
                out=krraw[:, ds(c * TL, TL)], in_=gout_kv[c, 0:64, RKV, :]
            )
        krsh = wb.tile([64, S], bf16, tag="krsh", bufs=1)
        nc.scalar.dma_start(out=krsh[0:32, :], in_=krraw[32:64, :])
        nc.scalar.dma_start(out=krsh[32:64, :], in_=krraw[0:32, :])
        kt1 = wb.tile([64, S], f32, tag="kt1", bufs=1)
        kt2 = wb.tile([64, S], f32, tag="kt2", bufs=1)
        nc.vector.tensor_mul(kt1, krraw, cos_sb[0:64, :])
        nc.vector.tensor_mul(kt2, krsh, sin_sb[0:64, :])
        nc.vector.tensor_sub(krope2_sb[0:32, :], kt1[0:32, :], kt2[0:32, :])
        nc.vector.tensor_add(krope2_sb[32:64, :], kt1[32:64, :], kt2[32:64, :])
        # fp8 copies into the packed score operand (rope half), one per head
        kr16 = krope2_sb[0:64, :].rearrange("p (k c) -> p k c", c=128)
        for m in range(NH):
            nc.vector.tensor_copy(kpack[0:64, m, :, 1, :], kr16)
        # k_nope: weight (j, m) loaded once, streamed over the 4 chunks
        for m in range(NH):
            knt = [sc_ps.tile([128, 1024], f32, tag="sc", name="knt") for _ in range(2)]
            for j in range(RKV):
                for c in range(NT):
                    nc.tensor.matmul(
                        knt[c // 2][:, ds((c % 2) * 512, 512)],
                        wkvuk_sb[:, j, ds(m * 128, 128)], kvg[:, j, c, :],
                        start=(j == 0), stop=(j == RKV - 1),
                    )
            for c in range(NT):
                nc.vector.tensor_copy(
                    kpack[:, m, ds(4 * c, 4), 0, :],
                    knt[c // 2][:, ds((c % 2) * 512, 512)],
                )
        # v: data-stationary per 128-token block (pv/aux banks)
        for s2 in range(S // 128):
            vp = (pv_ps if s2 % 2 == 0 else aux_ps).tile(
                [128, 512], f32, tag=("pv" if s2 % 2 == 0 else "aux"), name="vps"
            )
            for j in range(RKV):
                nc.tensor.matmul(
                    vp, kvg[:, j, s2 // 4, ds((s2 % 4) * 128, 128)], wkvuv_sb[:, j, :],
                    start=(j == 0), stop=(j == RKV - 1),
                )
            nc.vector.tensor_copy(v_sb[:, s2, :], vp)

        # ================= phase A2: q up-projections ====
        for c in range(NT):
            nc.scalar.dma_start(out=qlg[:, :, c, :], in_=gout_q[c])
        # wout lands after AG_q completes (gated by the qlg readback above on
        # the same queue) so its 2MB doesn't steal SDMA from the collective
        nc.scalar.dma_start(out=wout_sb[:], in_=wout_ap[:])
        # rope out-blocks first, chunk-local accumulators (pipelined with the
        # per-chunk rope math on DVE)
        for c in range(NT):
            csl = ds(c * T, T)
            ps1 = pv_ps.tile([128, T], f32, tag="pv", name="rp1")
            ps2 = aux_ps.tile([128, T], f32, tag="aux", name="rp2")
            for rr in range(RQ // 2):
                nc.tensor.matmul(
                    ps1, wqu_sb[:, ds(2 * rr, 2), ds(NH * D_NOPE, 128)],
                    qlg[:, ds(2 * rr, 2), c, :],
                    start=(rr == 0), stop=(rr == RQ // 2 - 1),
                    perf_mode=mybir.MatmulPerfMode.DoubleRow,
                )
                nc.tensor.matmul(
                    ps2, wqu_sb[:, ds(2 * rr, 2), ds(NH * D_NOPE + 128, 128)],
                    qlg[:, ds(2 * rr, 2), c, :],
                    start=(rr == 0), stop=(rr == RQ // 2 - 1),
                    perf_mode=mybir.MatmulPerfMode.DoubleRow,
                )
            qa = wb.tile([128, T], f32, tag="qa", bufs=1)
            qb = wb.tile([128, T], f32, tag="qb", bufs=1)
            nc.vector.tensor_mul(qa, ps1, cos_sb[:, csl])
            nc.vector.tensor_mul(qb, ps2, sin_sb[:, csl])
            y1 = wb.tile([128, T], f8, tag="y1", bufs=2)
            nc.vector.tensor_sub(y1, qa, qb)
            qa2 = wb.tile([128, T], f32, tag="qa", bufs=1)
            qb2 = wb.tile([128, T], f32, tag="qb", bufs=1)
            nc.vector.tensor_mul(qa2, ps2, cos_sb[:, csl])
            nc.vector.tensor_mul(qb2, ps1, sin_sb[:, csl])
            y2 = wb.tile([128, T], f8, tag="y2", bufs=2)
            nc.vector.tensor_add(y2, qa2, qb2)
            # assemble per-head [y1(32); y2(32)] rope rows (64-127 stay zero)
            for h in range(NH):
                nc.sync.dma_start(out=qpack[0:32, h, 1, csl], in_=y1[ds(32 * h, 32), :])
                nc.sync.dma_start(out=qpack[32:64, h, 1, csl], in_=y2[ds(32 * h, 32), :])
        # q_nope out-blocks: weight (r, m) streamed over the 4 chunks
        for m in range(NH):
            if m % 2 == 0:
                qnt = [
                    sc_ps.tile([128, 1024], f32, tag="sc", name="qnt") for _ in range(2)
                ]
                slots = [qnt[0][:, 0:512], qnt[0][:, 512:1024],
                         qnt[1][:, 0:512], qnt[1][:, 512:1024]]
            else:
                slots = [
                    pv_ps.tile([128, TL], f32, tag="pv", name="qn0"),
                    pv_ps.tile([128, TL], f32, tag="pv", name="qn1"),
                    aux_ps.tile([128, TL], f32, tag="aux", name="qn2"),
                    aux_ps.tile([128, TL], f32, tag="aux", name="qn3"),
                ]
            for rr in range(RQ // 2):
                for c in range(NT):
                    nc.tensor.matmul(
                        slots[c], wqu_sb[:, ds(2 * rr, 2), ds(m * 128, 128)],
                        qlg[:, ds(2 * rr, 2), c, :],
                        start=(rr == 0), stop=(rr == RQ // 2 - 1),
                        perf_mode=mybir.MatmulPerfMode.DoubleRow,
                    )
            for c in range(NT):
                nc.vector.tensor_copy(qpack[:, m, 0, ds(c * T, T)], slots[c])

        # ================= phase B + C: attention with interleaved out-proj ==
        wb.release()
        gat.release()
        wc = tc.alloc_tile_pool(name="wc", bufs=2)

        st = {}

        def sc_half(state, idx):
            # rotate [128,1024] sc tiles, handing out 512-wide halves
            if idx % 2 == 0:
                state["t"] = sc_ps.tile([128, 1024], f32, tag="sc", name="sct")
            return state["t"][:, ds((idx % 2) * 512, 512)]

        norm_pend = []

        def drain_norm(stn):
            h_, qsl_, pv_, den_ = stn
            rec = wc.tile([128, QC], f32, tag="rec", bufs=2)
            nc.vector.reciprocal_approx_fast(out=rec, in_=den_)
            nc.vector.tensor_mul(at_sb[:, h_, qsl_], pv_, rec)

        for qc in range(NQC):
            qsl = ds(qc * QC, QC)
            qb = qc * QC
            nfull = 2 * qc          # full (strictly below-diagonal) pairs
            for h in range(NH):
                pv = pv_ps.tile([128, QC], f32, tag="pv")
                den_ps = aux_ps.tile([128, QC], f32, tag="aux", name="den_ps")
                pend = []

                ehq = {}

                def flush_pair():
                    pt, pE, pEh = pend.pop(0)
                    nc.tensor.matmul(
                        pv, v_sb[:, 2 * pt, ds(h * V_DIM, V_DIM)], pE[:, 0:512],
                        start=(pt == 0), stop=False,
                    )
                    nc.tensor.matmul(
                        pv, v_sb[:, 2 * pt + 1, ds(h * V_DIM, V_DIM)], pE[:, 512:1024],
                        start=False, stop=False,
                    )
                    # one den matmul per QUAD: sum two pairs' Eh on DVE first
                    if pt % 2 == 0:
                        ehq["h"] = pEh
                    else:
                        q2 = wc.tile([128, QC], bf16, tag="Eh", bufs=6)
                        nc.vector.tensor_add(q2, ehq["h"], pEh)
                        nc.tensor.matmul(
                            den_ps, ones_sb[:], q2, start=(pt == 1), stop=False,
                        )

                for t in range(nfull):
                    kcA, kcB = 2 * t, 2 * t + 1
                    sct = sc_ps.tile([128, 1024], f32, tag="sc", name="sct_b")
                    # nope+rope fused: fp8 DoubleRow over the packed k-pair
                    nc.tensor.matmul(
                        sct[:, 0:512], kpack[:, h, kcA, :, :], qpack[:, h, :, qsl],
                        start=True, stop=True,
                        perf_mode=mybir.MatmulPerfMode.DoubleRow,
                    )
                    nc.tensor.matmul(
                        sct[:, 512:1024], kpack[:, h, kcB, :, :], qpack[:, h, :, qsl],
                        start=True, stop=True,
                        perf_mode=mybir.MatmulPerfMode.DoubleRow,
                    )
                    E = wc.tile([128, 1024], bf16, tag="E", bufs=6)
                    nc.scalar.activation(E, sct, AF.Exp, scale=SCALE)
                    Eh = wc.tile([128, QC], bf16, tag="Eh", bufs=6)
                    nc.vector.tensor_add(Eh, E[:, 0:512], E[:, 512:1024])
                    pend.append((t, E, Eh))
                    if len(pend) > 2:
                        flush_pair()

                # ---- diagonal band: 4 chunks at causal widths 512/384/256/128;
                # triangles zeroed on DVE after exp (no PE mask matmuls)
                offs = (0, 128, 256, 384)
                widths = (512, 384, 256, 128)
                cols = ((0, 0), (0, 512), (1, 0), (1, 512))  # (tile, col-base)
                dts = [
                    sc_ps.tile([128, 1024], f32, tag="sc", name="sct_d")
                    for _ in range(2)
                ]
                for d in range(4):
                    kc = 4 * qc + d
                    ti, cb = cols[d]
                    w = widths[d]
                    reg = dts[ti][:, ds(cb, w)]
                    nc.tensor.matmul(
                        reg, kpack[:, h, kc, :, :],
                        qpack[:, h, :, ds(qb + offs[d], w)],
                        start=True, stop=True,
                        perf_mode=mybir.MatmulPerfMode.DoubleRow,
                    )
                E1 = wc.tile([128, 1024], bf16, tag="E", bufs=6)
                nc.scalar.activation(E1[:, 0:896], dts[0][:, 0:896], AF.Exp, scale=SCALE)
                E2 = wc.tile([128, 1024], bf16, tag="E", bufs=6)
                nc.scalar.activation(E2[:, 0:640], dts[1][:, 0:640], AF.Exp, scale=SCALE)
                nc.vector.tensor_mul(E1[:, 0:128], E1[:, 0:128], tri_sb)
                nc.vector.tensor_mul(E1[:, 512:640], E1[:, 512:640], tri_sb)
                nc.vector.tensor_mul(E2[:, 0:128], E2[:, 0:128], tri_sb)
                nc.vector.tensor_mul(E2[:, 512:640], E2[:, 512:640], tri_sb)
                # per-query key-sums of the diagonal contributions
                Ehd = wc.tile([128, QC], bf16, tag="Eh", bufs=6)
                nc.vector.tensor_copy(Ehd, E1[:, 0:512])
                nc.vector.tensor_add(Ehd[:, 128:512], Ehd[:, 128:512], E1[:, 512:896])
                nc.vector.tensor_add(Ehd[:, 256:512], Ehd[:, 256:512], E2[:, 0:256])
                nc.vector.tensor_add(Ehd[:, 384:512], Ehd[:, 384:512], E2[:, 512:640])
                while pend:
                    flush_pair()
                Eref = ((E1, 0), (E1, 512), (E2, 0), (E2, 512))
                for d in range(4):
                    kc = 4 * qc + d
                    Et, cb = Eref[d]
                    nc.tensor.matmul(
                        pv[:, ds(offs[d], widths[d])],
                        v_sb[:, kc, ds(h * V_DIM, V_DIM)], Et[:, ds(cb, widths[d])],
                        start=(qc == 0 and d == 0), stop=(d == 3),
                    )
                nc.tensor.matmul(den_ps, ones_sb[:], Ehd, start=(qc == 0), stop=True)
                norm_pend.append((h, qsl, pv, den_ps))
                if len(norm_pend) > 1:
                    drain_norm(norm_pend.pop(0))
            while norm_pend:
                drain_norm(norm_pend.pop(0))
            # ---- out-projection for this qc's 4 token blocks ----
            for t16 in range(qc * 4, qc * 4 + 4):
                o_row = wc.tile([128, HID], f16, tag="ot", bufs=2)
                for n in range(HID // 512):
                    # rotate across sc halves AND pv tiles for a 6-deep psum
                    # rotation (hides the cast WAR)
                    if n < 2:
                        ps = sc_half(st, n)
                    else:
                        ps = pv_ps.tile([128, 512], f32, tag="pv", name="cps")
                    for f in range(NH):
                        nc.tensor.matmul(
                            ps, at_sb[:, f, ds(t16 * 128, 128)], wout_sb[:, f, ds(n * 512, 512)],
                            start=(f == 0), stop=(f == NH - 1),
                        )
                    nc.vector.tensor_copy(o_row[:, ds(n * 512, 512)], ps)
                nc.sync.dma_start(out=out_ap[ds(t16 * 128, 128), :], in_=o_row)

        wc.release()
        kvsb.release()
        w2.release()

    nc.compile()
    return nc


def get_nc():
    if "nc" not in _CACHE:
        _CACHE["nc"] = build_nc()
    return _CACHE["nc"]


def host_inputs(x, w_q_down, w_q_up, w_kv_down, kv_norm_w, w_kv_up, w_out):
    """Build the 8 per-core input shards (host-side prep, numpy only)."""
    bf = ml_dtypes.bfloat16
    x = np.asarray(x, np.float32)
    inv = 1.0 / THETA ** (np.arange(0, D_ROPE, 2, dtype=np.float64) / D_ROPE)
    ang = np.arange(S, dtype=np.float64)[:, None] * inv[None, :]      # (S, 32)
    cosq = np.ascontiguousarray(np.tile(np.cos(ang).T, (4, 1))).astype(bf)  # (128, S)
    sinq = np.ascontiguousarray(np.tile(np.sin(ang).T, (4, 1))).astype(bf)
    # 0/1 lower-triangle (valid where query >= key) for DVE masking
    r = np.arange(128)[:, None]
    j = np.arange(128)[None, :]
    trimask = (j >= r).astype(np.float32).astype(bf)
    ones128 = np.ones((128, 128), bf)
    wkv_eff = np.asarray(w_kv_up, np.float32) * np.asarray(kv_norm_w, np.float32)[:, None]

    def pmaj(w, *shape):
        # [K*128, N] -> partition-major [128, K, N] (-> optional extra reshape)
        kk = w.shape[0] // 128
        out = np.ascontiguousarray(w.reshape(kk, 128, w.shape[1]).transpose(1, 0, 2))
        return out.reshape(shape) if shape else out

    xT_bf = [np.ascontiguousarray(x[b].T).astype(bf) for b in range(B)]
    wqd_bf = np.asarray(w_q_down, np.float32).astype(bf)
    # wqd: [128, 6 rank-groups, 16 k-chunks, 256]
    wqd_pm = np.ascontiguousarray(
        wqd_bf.reshape(KH, 128, QG, 256).transpose(1, 2, 0, 3)
    )
    wkvd_pm = pmaj(np.asarray(w_kv_down, np.float32).astype(bf))
    wqu_f = np.asarray(w_q_up, np.float32)
    wout_f = np.asarray(w_out, np.float32)

    in_maps = []
    for ci in range(NCORES):
        b, hg = divmod(ci, 4)
        heads = list(range(NH * hg, NH * hg + NH))
        qu_cols = (
            [h * HEAD_DIM + j2 for h in heads for j2 in range(D_NOPE)]
            + [h * HEAD_DIM + D_NOPE + j2 for h in heads for j2 in range(32)]
            + [h * HEAD_DIM + D_NOPE + 32 + j2 for h in heads for j2 in range(32)]
        )
        kn_cols = [h * (D_NOPE + V_DIM) + j2 for h in heads for j2 in range(D_NOPE)]
        v_cols = [h * (D_NOPE + V_DIM) + D_NOPE + j2 for h in heads for j2 in range(V_DIM)]
        xq = np.ascontiguousarray(xT_bf[b][:, 512 * hg : 512 * (hg + 1)])
        in_maps.append(
            {
                "x": pmaj(xq),
                "wqd": wqd_pm,
                "wqu": pmaj(
                    np.ascontiguousarray(wqu_f[:, qu_cols]).astype(
                        ml_dtypes.float8_e4m3fn
                    )
                ),
                "wkvd": wkvd_pm,
                "wkvuk": pmaj(np.ascontiguousarray(wkv_eff[:, kn_cols]).astype(bf)),
                "wkvuv": pmaj(np.ascontiguousarray(wkv_eff[:, v_cols]).astype(bf)),
                "wout": pmaj(
                    np.ascontiguousarray(
                        wout_f[NH * V_DIM * hg : NH * V_DIM * (hg + 1), :]
                    ).astype(bf)
                ),
                "cosq": cosq,
                "sinq": sinq,
                "trimask": trimask,
                "ones128": ones128,
            }
        )
    return in_maps


def run(inputs, trace=False, trace_cores=None):
    from concourse.bass_utils import run_bass_kernel_spmd

    nc = get_nc()
    in_maps = host_inputs(**inputs)
    res = run_bass_kernel_spmd(
        nc,
        in_maps,
        core_ids=list(range(NCORES)),
        trace=trace,
        trace_cores=trace_cores,
    )
    out = np.zeros((B, S, HID), np.float32)
    for ci in range(NCORES):
        out[ci // 4] += res.results[ci]["out"].astype(np.float32)
    return out, res


def kernel(**inputs):
    out, _ = run(inputs, trace=False)
    return out


# revision 56
# speedup vs baseline: 1.1623x; 1.1623x over previous
"""MLA attention kernel for Trainium2 — 8-core tensor-parallel (self-contained).

Sharding: data-parallel over batch (2) x tensor-parallel over head groups
(4 groups of 4 heads) = 8 cores, SPMD (one NEFF, per-core input shards).
Core ci: batch ci//4, heads [4*(ci%4), 4*(ci%4)+4).

Layout/optimization highlights (~546us baseline -> ~440-460us):
  - every weight is host-pre-reshaped into its SBUF-resident partition-major
    layout so it loads as few large DMAs; first x/wkvd chunks land first so
    kv-down starts ~2us in
  - kv-down accumulates in the pv/aux PSUM banks and q-down in the sc banks,
    so the rmsnorm chain never blocks q-down and vice versa
  - rmsnorm: ACT Square reads the PSUM accumulators directly (parallel with
    the DVE bf16 cast), mean-square via ones-matmul, Sqrt table preloaded at
    kernel start -> the kv AllGather triggers as soon as the barrier clears
  - k-rope is gathered RAW and rotated post-gather, off the collective
    critical path; big wqu/wout weight DMAs are emitted after the collective
    triggers so the triggers' DMA-lane waits don't queue behind them
  - q latent is gathered in ONE fp8e4 AllGather (half the wire bytes);
    A1/A2 are weight-stationary over SBUF-resident all-chunk gathers
  - attention scores: fp8 DoubleRow matmuls over [nope(128) | rope(64+zero)]
    packed k/q operands (kpack/qpack) -> one matmul per 128-key chunk at
    2x rate; rope rows 64-127 of qpack are zeroed so kpack's garbage rows
    never contribute (NaN x 0 = NaN, so BOTH operands' pads are zeroed)
  - causal diagonal band computed at variable width (512/384/256/128) per
    128-key chunk; the leading 128-wide triangles are zeroed on DVE after
    exp (no PE mask matmuls); denominators via all-ones stationary matmul
  - out-projection interleaved per query chunk, output written fp16, final
    head-group reduction done host-side in fp32
  - total error (bf16 + fp8 q/score path) measured 1.26e-2 absmax-rel vs
    the 2e-2 gate; shadow-validated before hardware
"""

import math

import numpy as np
import ml_dtypes

# ---- problem constants (from the reference model) ----
B, S, HID = 2, 2048, 2048
H, D_NOPE, D_ROPE, V_DIM = 16, 128, 64, 128
KV_RANK, Q_RANK = 512, 1536
HEAD_DIM = D_NOPE + D_ROPE
THETA, EPS = 10000.0, 1e-6
NCORES = 8
NH = 4                    # heads per core
T = 512                   # token chunk
NT = S // T
QC = 512                  # attention query chunk
NQC = S // QC
KH = HID // 128           # 16 k-chunks over HID
RQ = Q_RANK // 128        # 12 chunks over q rank
RKV = KV_RANK // 128      # 4 chunks over kv rank
QG = 6                    # q-down rank groups (2 rank-chunks each)
Q_FP8 = True              # gather the q latent in fp8e4
SCALE = 1.0 / math.sqrt(HEAD_DIM)

_CACHE = {}


def build_nc():
    """Build the Bass/Tile program (one NeuronCore, run SPMD on 8)."""
    from contextlib import ExitStack

    import concourse.mybir as mybir
    import concourse.tile as tile
    from concourse import bacc
    from concourse.bass import ds

    dt = mybir.dt
    AF = mybir.ActivationFunctionType
    bf16 = dt.bfloat16
    f32 = dt.float32
    f16 = dt.float16
    f8 = dt.float8e4 if Q_FP8 else dt.bfloat16

    nc = bacc.Bacc(
        "TRN2",
        target_bir_lowering=False,
        debug=False,
        enable_asserts=False,
        num_devices=NCORES,
    )

    TL = S // 4  # local token quarter

    # ---- I/O (all partition-major, host pre-reshaped) ----
    x_ap = nc.dram_tensor("x", [128, KH, TL], bf16, kind="ExternalInput").ap()
    wqd_ap = nc.dram_tensor("wqd", [128, QG, KH, 256], bf16, kind="ExternalInput").ap()
    wqu_ap = nc.dram_tensor("wqu", [128, RQ, NH * HEAD_DIM], f8, kind="ExternalInput").ap()
    wkvd_ap = nc.dram_tensor("wkvd", [128, KH, KV_RANK + D_ROPE], bf16, kind="ExternalInput").ap()
    wkvuk_ap = nc.dram_tensor("wkvuk", [128, RKV, NH * D_NOPE], bf16, kind="ExternalInput").ap()
    wkvuv_ap = nc.dram_tensor("wkvuv", [128, RKV, NH * V_DIM], bf16, kind="ExternalInput").ap()
    wout_ap = nc.dram_tensor("wout", [128, NH, HID], bf16, kind="ExternalInput").ap()
    cos_ap = nc.dram_tensor("cosq", [128, S], bf16, kind="ExternalInput").ap()
    sin_ap = nc.dram_tensor("sinq", [128, S], bf16, kind="ExternalInput").ap()
    tri_ap = nc.dram_tensor("trimask", [128, 128], bf16, kind="ExternalInput").ap()
    ones_ap = nc.dram_tensor("ones128", [128, 128], bf16, kind="ExternalInput").ap()
    out_ap = nc.dram_tensor("out", [S, HID], f16, kind="ExternalOutput").ap()

    with tile.TileContext(nc) as tc, ExitStack() as ctx:
        # ---- PSUM pools: 2x[128,1024] + 2x[128,512] + 2x[128,512] = 8 banks
        sc_ps = ctx.enter_context(tc.tile_pool(name="sc_ps", bufs=2, space="PSUM"))
        pv_ps = ctx.enter_context(tc.tile_pool(name="pv_ps", bufs=2, space="PSUM"))
        aux_ps = ctx.enter_context(tc.tile_pool(name="aux_ps", bufs=2, space="PSUM"))

        const = ctx.enter_context(tc.tile_pool(name="const", bufs=1))
        woutp = ctx.enter_context(tc.tile_pool(name="woutp", bufs=1))
        dram = ctx.enter_context(tc.tile_pool(name="dram", bufs=1, space="DRAM"))

        # ---- up-projection weights (outlive w1 -> allocated below it)
        w2 = tc.alloc_tile_pool(name="w2", bufs=1)
        wkvuk_sb = w2.tile([128, RKV, NH * D_NOPE], bf16, tag="wkvuk")
        wkvuv_sb = w2.tile([128, RKV, NH * V_DIM], bf16, tag="wkvuv")
        wqu_sb = w2.tile([128, RQ, NH * HEAD_DIM], f8, tag="wqu")

        # ---- phase-A0 weights + x (released after A0)
        w1 = tc.alloc_tile_pool(name="w1", bufs=1)
        xt = w1.tile([128, KH, TL], bf16, tag="xt")
        wkvd_sb = w1.tile([128, KH, KV_RANK + D_ROPE], bf16, tag="wkvd")
        wqd_sb = w1.tile([128, QG, KH, 256], bf16, tag="wqd")

        # first chunks land first so kv-down starts immediately
        nc.sync.dma_start(out=xt[:, 0:2, :], in_=x_ap[:, 0:2, :])
        nc.scalar.dma_start(out=wkvd_sb[:, 0:2, :], in_=wkvd_ap[:, 0:2, :])
        nc.sync.dma_start(out=xt[:, 2:4, :], in_=x_ap[:, 2:4, :])
        nc.scalar.dma_start(out=wkvd_sb[:, 2:4, :], in_=wkvd_ap[:, 2:4, :])
        for q4 in range(1, 4):
            hk = ds(q4 * (KH // 4), KH // 4)
            nc.sync.dma_start(out=xt[:, hk, :], in_=x_ap[:, hk, :])
            nc.scalar.dma_start(out=wkvd_sb[:, hk, :], in_=wkvd_ap[:, hk, :])
        for g in range(QG):
            nc.sync.dma_start(out=wqd_sb[:, g, :, :], in_=wqd_ap[:, g, :, :])

        # ---- resident constants (scalar queue, small-first)
        ones_sb = const.tile([128, 128], bf16, name="ones_sb")
        nc.scalar.dma_start(out=ones_sb[:], in_=ones_ap[:])
        nc.scalar.dma_start(out=wkvuk_sb[:], in_=wkvuk_ap[:])
        nc.scalar.dma_start(out=wkvuv_sb[:], in_=wkvuv_ap[:])
        cos_sb = const.tile([128, S], bf16, name="cos_sb")
        nc.scalar.dma_start(out=cos_sb[:], in_=cos_ap[:])
        sin_sb = const.tile([128, S], bf16, name="sin_sb")
        nc.scalar.dma_start(out=sin_sb[:], in_=sin_ap[:])
        tri_sb = const.tile([128, 128], bf16, name="tri_sb")
        nc.scalar.dma_start(out=tri_sb[:], in_=tri_ap[:])
        wout_sb = woutp.tile([128, NH, HID], bf16, tag="wout")

        krope2_sb = const.tile([64, S], bf16, name="krope2_sb")
        at_sb = const.tile([128, NH, S], bf16, name="at_sb")
        eps_sb = const.tile([128, 1], f32, name="eps_sb")
        nc.gpsimd.memset(eps_sb[:], EPS)
        # preload the sqrt ACT table set off the critical path
        warm = const.tile([128, 1], f32, name="warm_sb")
        nc.scalar.activation(warm, eps_sb[:], AF.Sqrt)

        # DRAM bounce buffers for the latent gathers (partition-major; the
        # kv block 4 carries the row-duplicated rotated k-rope)
        gin_kv = dram.tile([128, RKV + 1, TL], bf16, name="gin_kv")
        gout_kv = dram.tile([4, 128, RKV + 1, TL], bf16, name="gout_kv")
        gin_q = dram.tile([128, RQ, TL], f8, name="gin_q")
        gout_q = dram.tile([4, 128, RQ, TL], f8, name="gout_q")
        GROUPS = [[0, 1, 2, 3], [4, 5, 6, 7]]

        # ================= phase A0: local down-projections =================
        # kv-down accumulates in pv/aux banks; krope+mean-square share one
        # sc tile's two banks; q-down rotates through the sc pool.
        wa = tc.alloc_tile_pool(name="wa", bufs=2)
        kv_ps = [
            pv_ps.tile([128, TL], f32, tag="pv", name="kvps0"),
            pv_ps.tile([128, TL], f32, tag="pv", name="kvps1"),
            aux_ps.tile([128, TL], f32, tag="aux", name="kvps2"),
            aux_ps.tile([128, TL], f32, tag="aux", name="kvps3"),
        ]
        a0t = sc_ps.tile([128, 1024], f32, tag="sc", name="a0t")
        krp_ps = a0t[0:64, 0:512]
        ms_ps = a0t[:, 512:1024]
        for k in range(KH):
            for j in range(RKV):
                nc.tensor.matmul(
                    kv_ps[j], wkvd_sb[:, k, ds(j * 128, 128)], xt[:, k, :],
                    start=(k == 0), stop=(k == KH - 1),
                )
            nc.tensor.matmul(
                krp_ps, wkvd_sb[:, k, ds(KV_RANK, D_ROPE)], xt[:, k, :],
                start=(k == 0), stop=(k == KH - 1),
            )
        # rmsnorm: bf16 copy (DVE) + square (ACT, direct from PSUM) -> ones
        # matmul -> Rsqrt (table already resident) -> scale
        kvc_bf = wa.tile([128, RKV, TL], bf16, tag="kvc", bufs=1)
        sq_bf = wa.tile([128, RKV, TL], bf16, tag="sq", bufs=1)
        for j in range(RKV):
            nc.vector.tensor_copy(kvc_bf[:, j, :], kv_ps[j])
            nc.scalar.activation(sq_bf[:, j, :], kv_ps[j], AF.Square)
        for j in range(RKV):
            nc.tensor.matmul(
                ms_ps, ones_sb[:], sq_bf[:, j, :],
                start=(j == 0), stop=(j == RKV - 1),
            )
        srt = wa.tile([128, TL], f32, tag="srt", bufs=1)
        nc.scalar.activation(srt, ms_ps, AF.Sqrt, bias=eps_sb[:], scale=1.0 / KV_RANK)
        rinv = wa.tile([128, TL], f32, tag="rinv", bufs=1)
        nc.vector.reciprocal_approx_fast(out=rinv, in_=srt)
        kvcn = wa.tile([128, RKV, TL], bf16, tag="kvcn", bufs=1)
        for j in range(RKV):
            nc.vector.tensor_mul(kvcn[:, j, :], kvc_bf[:, j, :], rinv)
        # k rope gathered RAW (rows 0-63 only); rotation happens post-gather,
        # off the collective critical path
        krb = wa.tile([64, TL], bf16, tag="krb", bufs=1)
        nc.vector.tensor_copy(krb, krp_ps)
        nc.scalar.dma_start(out=gin_kv[0:64, RKV, :], in_=krb[:])
        nc.scalar.dma_start(out=gin_kv[:, 0:RKV, :], in_=kvcn[:])
        nc.gpsimd.collective_compute(
            "AllGather", mybir.AluOpType.bypass, replica_groups=GROUPS,
            ins=[gin_kv.opt()], outs=[gout_kv.opt()],
        )

        # ---- q down: 6 groups of 2 rank-chunks, each on one sc tile; the
        # fp8 latent is staged per group and gathered in ONE AllGather ----
        qlat8 = wa.tile([128, RQ, TL], f8, tag="qlat8", bufs=1)
        for g in range(QG):
            qt = sc_ps.tile([128, 1024], f32, tag="sc", name="qdt")
            for k in range(KH):
                for m in range(2):
                    nc.tensor.matmul(
                        qt[:, ds(m * 512, 512)],
                        wqd_sb[:, g, k, ds(m * 128, 128)], xt[:, k, :],
                        start=(k == 0), stop=(k == KH - 1),
                    )
            for m in range(2):
                nc.vector.tensor_copy(qlat8[:, 2 * g + m, :], qt[:, ds(m * 512, 512)])
            nc.scalar.dma_start(
                out=gin_q[:, ds(2 * g, 2), :], in_=qlat8[:, ds(2 * g, 2), :]
            )
        nc.gpsimd.collective_compute(
            "AllGather", mybir.AluOpType.bypass, replica_groups=GROUPS,
            ins=[gin_q.opt()], outs=[gout_q.opt()],
        )
        # big late-needed weights AFTER the collective triggers, so the
        # trigger instructions' DMA-lane waits don't queue behind them
        nc.sync.dma_start(out=wqu_sb[:], in_=wqu_ap[:])

        wa.release()
        w1.release()
        # SBUF-resident intermediates (reuse w1's region)
        kvsb = tc.alloc_tile_pool(name="kvsb", bufs=1)
        # fp8 score operands, DoubleRow-packed: [.., kc, {nope,rope}, 128] /
        # [.., {nope,rope}, token]; rope rows 64-127 of qpack are ZERO so the
        # garbage rows of kpack's rope half never contribute
        kpack = kvsb.tile([128, NH, S // 128, 2, 128], f8, tag="kpack")
        qpack = kvsb.tile([128, NH, 2, S], f8, tag="qpack")
        v_sb = kvsb.tile([128, S // 128, NH * V_DIM], bf16, tag="v")
        nc.gpsimd.memset(qpack[64:128, :, 1, :], 0.0)
        nc.gpsimd.memset(kpack[64:128, :, :, 1, :], 0.0)
        gat = tc.alloc_tile_pool(name="gat", bufs=1)
        kvg = gat.tile([128, RKV, NT, T], bf16, tag="kvg")
        qlg = gat.tile([128, RQ, NT, T], f8, tag="qlg")
        wb = tc.alloc_tile_pool(name="wb", bufs=2)

        # ================= phase A1: kv up-projections (weight-stationary) ===
        for c in range(NT):
            nc.scalar.dma_start(out=kvg[:, :, c, :], in_=gout_kv[c, :, 0:RKV, :])
        # k-rope rotation for the full sequence (raw gathered halves):
        # [x1;x2] rows 0-63, shifted copy, rotate, duplicate to rows 64-127
        krraw = wb.tile([64, S], bf16, tag="krraw", bufs=1)
        for c in range(NT):
            nc.scalar.dma_start(
                out=krraw[:, ds(c * TL, TL)], in_=gout_kv[c, 0:64, RKV, :]
            )
        krsh = wb.tile([64, S], bf16, tag="krsh", bufs=1)
        nc.scalar.dma_start(out=krsh[0:32, :], in_=krraw[32:64, :])
        nc.scalar.dma_start(out=krsh[32:64, :], in_=krraw[0:32, :])
        kt1 = wb.tile([64, S], f32, tag="kt1", bufs=1)
        kt2 = wb.tile([64, S], f32, tag="kt2", bufs=1)
        nc.vector.tensor_mul(kt1, krraw, cos_sb[0:64, :])
        nc.vector.tensor_mul(kt2, krsh, sin_sb[0:64, :])
        nc.vector.tensor_sub(krope2_sb[0:32, :], kt1[0:32, :], kt2[0:32, :])
        nc.vector.tensor_add(krope2_sb[32:64, :], kt1[32:64, :], kt2[32:64, :])
        # fp8 copies into the packed score operand (rope half), one per head
        kr16 = krope2_sb[0:64, :].rearrange("p (k c) -> p k c", c=128)
        for m in range(NH):
            nc.vector.tensor_copy(kpack[0:64, m, :, 1, :], kr16)
        # k_nope: weight (j, m) loaded once, streamed over the 4 chunks
        for m in range(NH):
            knt = [sc_ps.tile([128, 1024], f32, tag="sc", name="knt") for _ in range(2)]
            for j in range(RKV):
                for c in range(NT):
                    nc.tensor.matmul(
                        knt[c // 2][:, ds((c % 2) * 512, 512)],
                        wkvuk_sb[:, j, ds(m * 128, 128)], kvg[:, j, c, :],
                        start=(j == 0), stop=(j == RKV - 1),
                    )
            for c in range(NT):
                nc.vector.tensor_copy(
                    kpack[:, m, ds(4 * c, 4), 0, :],
                    knt[c // 2][:, ds((c % 2) * 512, 512)],
                )
        # v: data-stationary per 128-token block (pv/aux banks)
        for s2 in range(S // 128):
            vp = (pv_ps if s2 % 2 == 0 else aux_ps).tile(
                [128, 512], f32, tag=("pv" if s2 % 2 == 0 else "aux"), name="vps"
            )
            for j in range(RKV):
                nc.tensor.matmul(
                    vp, kvg[:, j, s2 // 4, ds((s2 % 4) * 128, 128)], wkvuv_sb[:, j, :],
                    start=(j == 0), stop=(j == RKV - 1),
                )
            nc.vector.tensor_copy(v_sb[:, s2, :], vp)

        # ================= phase A2: q up-projections ====
        for c in range(NT):
            nc.scalar.dma_start(out=qlg[:, :, c, :], in_=gout_q[c])
        # wout lands after AG_q completes (gated by the qlg readback above on
        # the same queue) so its 2MB doesn't steal SDMA from the collective
        nc.scalar.dma_start(out=wout_sb[:], in_=wout_ap[:])
        # rope out-blocks first, chunk-local accumulators (pipelined with the
        # per-chunk rope math on DVE)
        for c in range(NT):
            csl = ds(c * T, T)
            ps1 = pv_ps.tile([128, T], f32, tag="pv", name="rp1")
            ps2 = aux_ps.tile([128, T], f32, tag="aux", name="rp2")
            for rr in range(RQ // 2):
                nc.tensor.matmul(
                    ps1, wqu_sb[:, ds(2 * rr, 2), ds(NH * D_NOPE, 128)],
                    qlg[:, ds(2 * rr, 2), c, :],
                    start=(rr == 0), stop=(rr == RQ // 2 - 1),
                    perf_mode=mybir.MatmulPerfMode.DoubleRow,
                )
                nc.tensor.matmul(
                    ps2, wqu_sb[:, ds(2 * rr, 2), ds(NH * D_NOPE + 128, 128)],
                    qlg[:, ds(2 * rr, 2), c, :],
                    start=(rr == 0), stop=(rr == RQ // 2 - 1),
                    perf_mode=mybir.MatmulPerfMode.DoubleRow,
                )
            qa = wb.tile([128, T], f32, tag="qa", bufs=1)
            qb = wb.tile([128, T], f32, tag="qb", bufs=1)
            nc.vector.tensor_mul(qa, ps1, cos_sb[:, csl])
            nc.vector.tensor_mul(qb, ps2, sin_sb[:, csl])
            y1 = wb.tile([128, T], f8, tag="y1", bufs=2)
            nc.vector.tensor_sub(y1, qa, qb)
            qa2 = wb.tile([128, T], f32, tag="qa", bufs=1)
            qb2 = wb.tile([128, T], f32, tag="qb", bufs=1)
            nc.vector.tensor_mul(qa2, ps2, cos_sb[:, csl])
            nc.vector.tensor_mul(qb2, ps1, sin_sb[:, csl])
            y2 = wb.tile([128, T], f8, tag="y2", bufs=2)
            nc.vector.tensor_add(y2, qa2, qb2)
            # assemble per-head [y1(32); y2(32)] rope rows (64-127 stay zero)
            for h in range(NH):
                nc.sync.dma_start(out=qpack[0:32, h, 1, csl], in_=y1[ds(32 * h, 32), :])
                nc.sync.dma_start(out=qpack[32:64, h, 1, csl], in_=y2[ds(32 * h, 32), :])
        # q_nope out-blocks: weight (r, m) streamed over the 4 chunks
        for m in range(NH):
            if m % 2 == 0:
                qnt = [
                    sc_ps.tile([128, 1024], f32, tag="sc", name="qnt") for _ in range(2)
                ]
                slots = [qnt[0][:, 0:512], qnt[0][:, 512:1024],
                         qnt[1][:, 0:512], qnt[1][:, 512:1024]]
            else:
                slots = [
                    pv_ps.tile([128, TL], f32, tag="pv", name="qn0"),
                    pv_ps.tile([128, TL], f32, tag="pv", name="qn1"),
                    aux_ps.tile([128, TL], f32, tag="aux", name="qn2"),
                    aux_ps.tile([128, TL], f32, tag="aux", name="qn3"),
                ]
            for rr in range(RQ // 2):
                for c in range(NT):
                    nc.tensor.matmul(
                        slots[c], wqu_sb[:, ds(2 * rr, 2), ds(m * 128, 128)],
                        qlg[:, ds(2 * rr, 2), c, :],
                        start=(rr == 0), stop=(rr == RQ // 2 - 1),
                        perf_mode=mybir.MatmulPerfMode.DoubleRow,
                    )
            for c in range(NT):
                nc.vector.tensor_copy(qpack[:, m, 0, ds(c * T, T)], slots[c])

        # ================= phase B + C: attention with interleaved out-proj ==
        wb.release()
        gat.release()
        wc = tc.alloc_tile_pool(name="wc", bufs=2)

        st = {}

        def sc_half(state, idx):
            # rotate [128,1024] sc tiles, handing out 512-wide halves
            if idx % 2 == 0:
                state["t"] = sc_ps.tile([128, 1024], f32, tag="sc", name="sct")
            return state["t"][:, ds((idx % 2) * 512, 512)]

        norm_pend = []

        def drain_norm(stn):
            h_, qsl_, pv_, den_ = stn
            rec = wc.tile([128, QC], f32, tag="rec", bufs=2)
            nc.vector.reciprocal_approx_fast(out=rec, in_=den_)
            nc.vector.tensor_mul(at_sb[:, h_, qsl_], pv_, rec)

        for qc in range(NQC):
            qsl = ds(qc * QC, QC)
            qb = qc * QC
            nfull = 2 * qc          # full (strictly below-diagonal) pairs
            for h in range(NH):
                pv = pv_ps.tile([128, QC], f32, tag="pv")
                den_ps = aux_ps.tile([128, QC], f32, tag="aux", name="den_ps")
                pend = []

                ehq = {}

                def flush_pair():
                    pt, pE, pEh = pend.pop(0)
                    nc.tensor.matmul(
                        pv, v_sb[:, 2 * pt, ds(h * V_DIM, V_DIM)], pE[:, 0:512],
                        start=(pt == 0), stop=False,
                    )
                    nc.tensor.matmul(
                        pv, v_sb[:, 2 * pt + 1, ds(h * V_DIM, V_DIM)], pE[:, 512:1024],
                        start=False, stop=False,
                    )
                    # one den matmul per QUAD: sum two pairs' Eh on DVE first
                    if pt % 2 == 0:
                        ehq["h"] = pEh
                    else:
                        q2 = wc.tile([128, QC], bf16, tag="Eh", bufs=6)
                        nc.vector.tensor_add(q2, ehq["h"], pEh)
                        nc.tensor.matmul(
                            den_ps, ones_sb[:], q2, start=(pt == 1), stop=False,
                        )

                for t in range(nfull):
                    kcA, kcB = 2 * t, 2 * t + 1
                    sct = sc_ps.tile([128, 1024], f32, tag="sc", name="sct_b")
                    # nope+rope fused: fp8 DoubleRow over the packed k-pair
                    nc.tensor.matmul(
                        sct[:, 0:512], kpack[:, h, kcA, :, :], qpack[:, h, :, qsl],
                        start=True, stop=True,
                        perf_mode=mybir.MatmulPerfMode.DoubleRow,
                    )
                    nc.tensor.matmul(
                        sct[:, 512:1024], kpack[:, h, kcB, :, :], qpack[:, h, :, qsl],
                        start=True, stop=True,
                        perf_mode=mybir.MatmulPerfMode.DoubleRow,
                    )
                    E = wc.tile([128, 1024], bf16, tag="E", bufs=8)
                    nc.scalar.activation(E, sct, AF.Exp, scale=SCALE)
                    Eh = wc.tile([128, QC], bf16, tag="Eh", bufs=6)
                    nc.vector.tensor_add(Eh, E[:, 0:512], E[:, 512:1024])
                    pend.append((t, E, Eh))
                    if len(pend) > 2:
                        flush_pair()

                # ---- diagonal band: 4 chunks at causal widths 512/384/256/128;
                # triangles zeroed on DVE after exp (no PE mask matmuls)
                offs = (0, 128, 256, 384)
                widths = (512, 384, 256, 128)
                cols = ((0, 0), (0, 512), (1, 0), (1, 512))  # (tile, col-base)
                dts = [
                    sc_ps.tile([128, 1024], f32, tag="sc", name="sct_d")
                    for _ in range(2)
                ]
                for d in range(4):
                    kc = 4 * qc + d
                    ti, cb = cols[d]
                    w = widths[d]
                    reg = dts[ti][:, ds(cb, w)]
                    nc.tensor.matmul(
                        reg, kpack[:, h, kc, :, :],
                        qpack[:, h, :, ds(qb + offs[d], w)],
                        start=True, stop=True,
                        perf_mode=mybir.MatmulPerfMode.DoubleRow,
                    )
                E1 = wc.tile([128, 1024], bf16, tag="E", bufs=8)
                nc.scalar.activation(E1[:, 0:896], dts[0][:, 0:896], AF.Exp, scale=SCALE)
                E2 = wc.tile([128, 1024], bf16, tag="E", bufs=8)
                nc.scalar.activation(E2[:, 0:640], dts[1][:, 0:640], AF.Exp, scale=SCALE)
                nc.vector.tensor_mul(E1[:, 0:128], E1[:, 0:128], tri_sb)
                nc.vector.tensor_mul(E1[:, 512:640], E1[:, 512:640], tri_sb)
                nc.vector.tensor_mul(E2[:, 0:128], E2[:, 0:128], tri_sb)
                nc.vector.tensor_mul(E2[:, 512:640], E2[:, 512:640], tri_sb)
                # per-query key-sums of the diagonal contributions
                Ehd = wc.tile([128, QC], bf16, tag="Eh", bufs=6)
                nc.vector.tensor_copy(Ehd, E1[:, 0:512])
                nc.vector.tensor_add(Ehd[:, 128:512], Ehd[:, 128:512], E1[:, 512:896])
                nc.vector.tensor_add(Ehd[:, 256:512], Ehd[:, 256:512], E2[:, 0:256])
                nc.vector.tensor_add(Ehd[:, 384:512], Ehd[:, 384:512], E2[:, 512:640])
                while pend:
                    flush_pair()
                Eref = ((E1, 0), (E1, 512), (E2, 0), (E2, 512))
                for d in range(4):
                    kc = 4 * qc + d
                    Et, cb = Eref[d]
                    nc.tensor.matmul(
                        pv[:, ds(offs[d], widths[d])],
                        v_sb[:, kc, ds(h * V_DIM, V_DIM)], Et[:, ds(cb, widths[d])],
                        start=(qc == 0 and d == 0), stop=(d == 3),
                    )
                nc.tensor.matmul(den_ps, ones_sb[:], Ehd, start=(qc == 0), stop=True)
                norm_pend.append((h, qsl, pv, den_ps))
                if len(norm_pend) > 1:
                    drain_norm(norm_pend.pop(0))
            while norm_pend:
                drain_norm(norm_pend.pop(0))
            # ---- out-projection for this qc's 4 token blocks ----
            for t16 in range(qc * 4, qc * 4 + 4):
                o_row = wc.tile([128, HID], f16, tag="ot", bufs=2)
                for n in range(HID // 512):
                    # rotate across sc halves AND pv tiles for a 6-deep psum
                    # rotation (hides the cast WAR)
                    if n < 2:
                        ps = sc_half(st, n)
                    else:
                        ps = pv_ps.tile([128, 512], f32, tag="pv", name="cps")
                    for f in range(NH):
                        nc.tensor.matmul(
                            ps, at_sb[:, f, ds(t16 * 128, 128)], wout_sb[:, f, ds(n * 512, 512)],
                            start=(f == 0), stop=(f == NH - 1),
                        )
                    # ACT has slack in the attention window; keep DVE free
                    nc.scalar.copy(o_row[:, ds(n * 512, 512)], ps)
                nc.sync.dma_start(out=out_ap[ds(t16 * 128, 128), :], in_=o_row)

        wc.release()
        kvsb.release()
        w2.release()

    nc.compile()
    return nc


def get_nc():
    if "nc" not in _CACHE:
        _CACHE["nc"] = build_nc()
    return _CACHE["nc"]


def host_inputs(x, w_q_down, w_q_up, w_kv_down, kv_norm_w, w_kv_up, w_out):
    """Build the 8 per-core input shards (host-side prep, numpy only)."""
    bf = ml_dtypes.bfloat16
    x = np.asarray(x, np.float32)
    inv = 1.0 / THETA ** (np.arange(0, D_ROPE, 2, dtype=np.float64) / D_ROPE)
    ang = np.arange(S, dtype=np.float64)[:, None] * inv[None, :]      # (S, 32)
    cosq = np.ascontiguousarray(np.tile(np.cos(ang).T, (4, 1))).astype(bf)  # (128, S)
    sinq = np.ascontiguousarray(np.tile(np.sin(ang).T, (4, 1))).astype(bf)
    # 0/1 lower-triangle (valid where query >= key) for DVE masking
    r = np.arange(128)[:, None]
    j = np.arange(128)[None, :]
    trimask = (j >= r).astype(np.float32).astype(bf)
    ones128 = np.ones((128, 128), bf)
    wkv_eff = np.asarray(w_kv_up, np.float32) * np.asarray(kv_norm_w, np.float32)[:, None]

    def pmaj(w, *shape):
        # [K*128, N] -> partition-major [128, K, N] (-> optional extra reshape)
        kk = w.shape[0] // 128
        out = np.ascontiguousarray(w.reshape(kk, 128, w.shape[1]).transpose(1, 0, 2))
        return out.reshape(shape) if shape else out

    xT_bf = [np.ascontiguousarray(x[b].T).astype(bf) for b in range(B)]
    wqd_bf = np.asarray(w_q_down, np.float32).astype(bf)
    # wqd: [128, 6 rank-groups, 16 k-chunks, 256]
    wqd_pm = np.ascontiguousarray(
        wqd_bf.reshape(KH, 128, QG, 256).transpose(1, 2, 0, 3)
    )
    wkvd_pm = pmaj(np.asarray(w_kv_down, np.float32).astype(bf))
    wqu_f = np.asarray(w_q_up, np.float32)
    wout_f = np.asarray(w_out, np.float32)

    in_maps = []
    for ci in range(NCORES):
        b, hg = divmod(ci, 4)
        heads = list(range(NH * hg, NH * hg + NH))
        qu_cols = (
            [h * HEAD_DIM + j2 for h in heads for j2 in range(D_NOPE)]
            + [h * HEAD_DIM + D_NOPE + j2 for h in heads for j2 in range(32)]
            + [h * HEAD_DIM + D_NOPE + 32 + j2 for h in heads for j2 in range(32)]
        )
        kn_cols = [h * (D_NOPE + V_DIM) + j2 for h in heads for j2 in range(D_NOPE)]
        v_cols = [h * (D_NOPE + V_DIM) + D_NOPE + j2 for h in heads for j2 in range(V_DIM)]
        xq = np.ascontiguousarray(xT_bf[b][:, 512 * hg : 512 * (hg + 1)])
        in_maps.append(
            {
                "x": pmaj(xq),
                "wqd": wqd_pm,
                "wqu": pmaj(
                    np.ascontiguousarray(wqu_f[:, qu_cols]).astype(
                        ml_dtypes.float8_e4m3fn
                    )
                ),
                "wkvd": wkvd_pm,
                "wkvuk": pmaj(np.ascontiguousarray(wkv_eff[:, kn_cols]).astype(bf)),
                "wkvuv": pmaj(np.ascontiguousarray(wkv_eff[:, v_cols]).astype(bf)),
                "wout": pmaj(
                    np.ascontiguousarray(
                        wout_f[NH * V_DIM * hg : NH * V_DIM * (hg + 1), :]
                    ).astype(bf)
                ),
                "cosq": cosq,
                "sinq": sinq,
                "trimask": trimask,
                "ones128": ones128,
            }
        )
    return in_maps


def run(inputs, trace=False, trace_cores=None):
    from concourse.bass_utils import run_bass_kernel_spmd

    nc = get_nc()
    in_maps = host_inputs(**inputs)
    res = run_bass_kernel_spmd(
        nc,
        in_maps,
        core_ids=list(range(NCORES)),
        trace=trace,
        trace_cores=trace_cores,
    )
    out = np.zeros((B, S, HID), np.float32)
    for ci in range(NCORES):
        out[ci // 4] += res.results[ci]["out"].astype(np.float32)
    return out, res


def kernel(**inputs):
    out, _ = run(inputs, trace=False)
    return out
